# revision 26
# baseline (speedup 1.0000x reference)
"""Bahdanau additive attention on 8 Trainium2 NeuronCores.

Problem shapes (hardcoded): B=4, T=128, S=512, H=256, fp32.

Sharding: data-parallel over (batch, T-half): core c handles b = c//2,
t in [64*(c%2), 64*(c%2)+64).  Same SPMD program on every core; weights
replicated.  No collectives.

Algorithm: the additive-attention score
    e[t,s] = sum_h v[h] * tanh(pq[t,h] + pe[s,h])
is evaluated through a separable expansion instead of materializing the
(T,S,H) tensor.  With a = tanh(pq), w = tanh(pe):
    tanh(x+y) - tanh(x) ~= sum_{p=1..6} a^{p-1}(c0_p + c1_p a^2) w^p
(trimmed-minimax fit against the end-to-end output error; the tanh(x)
term is constant over s and drops under softmax shift-invariance).
Each term is a rank-1 update in (t,s) contracted over h, so e becomes
12 dense matmul passes accumulated in one PSUM bank, plus a rank-1
mask pass (single-partition ones x mask row) placed FIRST so the last
thing before exp is a data pass.  Ladder: w2=w*w, w3=w2*w, w5=w2*w3 on
Vector; w4=Sq(w2), w6=Sq(w3) on Scalar.  A-chain G_p on Vector, At/M
products on GpSimd (M1/M5/M6 on Vector to hit stream deadlines).
Inputs stream over both HWDGE DMA queues (Sync + Scalar), most-urgent
first.  Softmax tail: split exp with accum columns, PE transposes of
alpha^T, context accumulated TRANSPOSED (c^T = sum_s enc_sc^T @ ptT_sc)
so no extra c transpose is needed; out = tanh(r*attn_c + attn_q) via
one fused scalar_tensor_tensor.  fp16 throughout, fp32 PSUM.
"""

import numpy as np

B, T, S, H = 4, 128, 512, 256
TLOC = 64
NCORES = 8
P = 128
HC = H // P        # 2 h-chunks

# trimmed-minimax fit (fit_v4.py J6_123456_std): emulated p99.99 = 1.64e-2
POWERS = [1, 2, 3, 4, 5, 6]
CF = {}

_CF_KEYS = [(1, 0), (1, 2), (2, 1), (2, 3), (3, 2), (3, 4), (4, 3), (4, 5),
            (5, 4), (5, 6), (6, 5), (6, 7)]


def _load_cf():
    # coefficients produced by fit_v4.py (J6_123456_std), hardcoded
    vals = [1.024896261659279, -0.896439120239331,
            -1.2593520033723025, 0.3385575705136777,
            0.1743475243113486, -1.1750131757880733,
            2.270998990133468, 1.3712367736362034,
            4.101531713287659, -2.78572455103627,
            -8.596690258997429, 5.413507705507528]
    for (p, d), c in zip(_CF_KEYS, vals):
        CF.setdefault(p, [0.0, 0.0])
        CF[p][0 if d == p - 1 else 1] = c


_load_cf()

_CACHE = {}


def build_module():
    if "nc" in _CACHE:
        return _CACHE["nc"]

    try:
        import concourse.bass  # noqa: F401
    except ImportError:
        import sys
        sys.path.insert(0, "/opt/trn_rl_repo")

    import concourse.tile as tile
    from concourse import bacc, mybir

    f32 = mybir.dt.float32
    f16 = mybir.dt.float16
    AF = mybir.ActivationFunctionType
    ALU = mybir.AluOpType

    nc = bacc.Bacc(
        "TRN2",
        target_bir_lowering=False,
        debug=False,
        enable_asserts=False,
        num_devices=NCORES,
    )

    # packed fp16 inputs, split across the two HWDGE queues by urgency:
    #  Sync  : pk_a [encT (1024) | whT (512)]      gates pe -> ladder
    #          pk_c [ctx enc (1024)]               needed late (context)
    #  Scalar: pk_b [qT 128 | wsT 512 | v 128 | I 64]  gates pq -> A-chain
    #          pk_d [woutT (1024)]                 needed mid/late
    #  pk_m  : mask row (1, 512), rank-1 mask pass
    d_pa = nc.dram_tensor("pack_a", (P, 1536), f16, kind="ExternalInput").ap()
    d_pb = nc.dram_tensor("pack_b", (P, 832), f16, kind="ExternalInput").ap()
    d_pc = nc.dram_tensor("pack_c", (P, 1024), f16, kind="ExternalInput").ap()
    d_pd = nc.dram_tensor("pack_d", (P, 1024), f16, kind="ExternalInput").ap()
    d_pm = nc.dram_tensor("pack_m", (32, 512), f16, kind="ExternalInput").ap()
    d_out = nc.dram_tensor("out_l", (TLOC, H), f32, kind="ExternalOutput").ap()

    with tile.TileContext(nc) as tc:
        from contextlib import ExitStack

        with ExitStack() as ctx:
            consts = ctx.enter_context(tc.tile_pool(name="consts", bufs=1))
            bpow = ctx.enter_context(tc.tile_pool(name="bpow", bufs=1))
            asm = ctx.enter_context(tc.tile_pool(name="asm", bufs=1))
            tailp = ctx.enter_context(tc.tile_pool(name="tailp", bufs=1))
            psA = ctx.enter_context(tc.tile_pool(name="psA", bufs=1, space="PSUM"))
            psB = ctx.enter_context(tc.tile_pool(name="psB", bufs=1, space="PSUM"))
            psQ = ctx.enter_context(tc.tile_pool(name="psQ", bufs=1, space="PSUM"))
            psE = ctx.enter_context(tc.tile_pool(name="psE", bufs=1, space="PSUM"))
            psT = ctx.enter_context(tc.tile_pool(name="psT", bufs=3, space="PSUM"))

            pa = consts.tile([P, 1536], f16)
            nc.sync.dma_start(pa[:], d_pa[:, :])
            pb = consts.tile([P, 832], f16)
            nc.scalar.dma_start(pb[:], d_pb[:, :])
            pm_t = consts.tile([P, 512], f16)
            pm = pm_t[0:32, :]
            nc.sync.dma_start(pm, d_pm[:, :])
            pc = consts.tile([P, 1024], f16)
            nc.sync.dma_start(pc[:], d_pc[:, :])
            pd = consts.tile([P, 1024], f16)
            nc.scalar.dma_start(pd[:], d_pd[:, :])

            encT = [pa[:, 0:512], pa[:, 512:1024]]          # (h-chunk, s)
            wh_sb = [pa[:, 1024 + kc * H:1024 + (kc + 1) * H] for kc in range(HC)]
            qT = pb[:, 0:128]                               # [hc0 t | hc1 t]
            ws_sb = [pb[:, 128 + kc * H:128 + (kc + 1) * H] for kc in range(HC)]
            vbc = pb[:, 640:768]
            ident = pb[:, 768:832]                          # rows 0:64 = I64
            ctxenc = pc[:, 0:1024]                          # 4 x (128 x 256)
            wout_sb = [pd[:, fc * H:(fc + 1) * H] for fc in range(4)]

            neg4 = consts.tile([TLOC, 1], f32)
            nc.vector.memset(neg4[:], -4.0)

            ones1_t = consts.tile([P, TLOC], f16)
            ones1 = ones1_t[0:32, :]
            nc.gpsimd.memset(ones1, 1.0)

            # ---- projections (PE): pe_ps0 first (pack_a lands first),
            # then pq (pack_b), then pe_ps1 -- so tanh0 and the A-chain
            # both start as early as possible.
            pe_ps = [psA.tile([P, 512], f32, name="pe_ps0"),
                     psB.tile([P, 512], f32, name="pe_ps1")]
            pq_ps = psQ.tile([P, 128], f32, name="pq_ps")
            for kc in range(HC):
                nc.tensor.matmul(
                    pe_ps[0][:],
                    lhsT=wh_sb[kc][:, 0:P],
                    rhs=encT[kc][:],
                    start=(kc == 0), stop=(kc == HC - 1),
                )
            for oc in range(HC):
                for kc in range(HC):
                    nc.tensor.matmul(
                        pq_ps[:, oc * TLOC:(oc + 1) * TLOC],
                        lhsT=ws_sb[kc][:, oc * P:(oc + 1) * P],
                        rhs=qT[:, kc * TLOC:(kc + 1) * TLOC],
                        start=(kc == 0), stop=(kc == HC - 1),
                    )
            for kc in range(HC):
                nc.tensor.matmul(
                    pe_ps[1][:],
                    lhsT=wh_sb[kc][:, P:2 * P],
                    rhs=encT[kc][:],
                    start=(kc == 0), stop=(kc == HC - 1),
                )

            # ---- base activations ----
            alpha = asm.tile([P, 128], f16, name="alpha")
            a2 = asm.tile([P, 128], f16, name="a2")
            w1 = bpow.tile([P, 1024], f16, name="w1")
            nc.scalar.activation(w1[:, 0:512], pe_ps[0][:], AF.Tanh)
            with tc.high_priority():
                nc.scalar.activation(alpha[:], pq_ps[:], AF.Tanh)
                nc.vector.tensor_tensor(out=a2[:], in0=alpha[:], in1=alpha[:],
                                        op=ALU.mult)
            nc.scalar.activation(w1[:, 512:1024], pe_ps[1][:], AF.Tanh)

            # ---- A-side: G_p = c0 + c1 a^2 (Vector), At_k = v a^k chain
            #      (GpSimd), M_p = At_{p-1} G_p (GpSimd; 1,5,6 on Vector)
            G = {}
            M = {}
            for p in POWERS:
                G[p] = asm.tile([P, 128], f16, name=f"G{p}")
                M[p] = asm.tile([P, 128], f16, name=f"M{p}")
            At = {0: vbc}
            for k in (1, 2, 3, 4, 5):
                At[k] = asm.tile([P, 128], f16, name=f"At{k}")

            V = nc.vector
            GP = nc.gpsimd

            def mk_g(p):
                c0, c1 = CF[p]
                V.tensor_scalar(G[p][:], a2[:], float(c1), float(c0),
                                ALU.mult, ALU.add)

            with tc.high_priority():
                mk_g(1)
                V.tensor_tensor(out=M[1][:], in0=vbc[:], in1=G[1][:],
                                op=ALU.mult)
            GP.tensor_tensor(out=At[1][:], in0=vbc[:], in1=alpha[:],
                             op=ALU.mult)
            GP.tensor_tensor(out=At[2][:], in0=vbc[:], in1=a2[:], op=ALU.mult)
            Wt = {1: w1}
            for p in POWERS[1:]:
                Wt[p] = bpow.tile([P, 1024], f16, name=f"w{p}")

            def vmul(dst, sa, sb, hc):
                V.tensor_tensor(out=Wt[dst][:, hc * 512:(hc + 1) * 512],
                                in0=Wt[sa][:, hc * 512:(hc + 1) * 512],
                                in1=Wt[sb][:, hc * 512:(hc + 1) * 512],
                                op=ALU.mult)

            def ssq(dst, src, hc):
                nc.scalar.activation(Wt[dst][:, hc * 512:(hc + 1) * 512],
                                     Wt[src][:, hc * 512:(hc + 1) * 512],
                                     AF.Square)

            for p in (2, 3, 4, 5, 6):
                mk_g(p)
            GP.tensor_tensor(out=M[2][:], in0=At[1][:], in1=G[2][:],
                             op=ALU.mult)
            GP.tensor_tensor(out=At[3][:], in0=At[1][:], in1=a2[:],
                             op=ALU.mult)
            GP.tensor_tensor(out=M[3][:], in0=At[2][:], in1=G[3][:],
                             op=ALU.mult)
            GP.tensor_tensor(out=At[4][:], in0=At[2][:], in1=a2[:],
                             op=ALU.mult)
            GP.tensor_tensor(out=M[4][:], in0=At[3][:], in1=G[4][:],
                             op=ALU.mult)
            GP.tensor_tensor(out=At[5][:], in0=At[3][:], in1=a2[:],
                             op=ALU.mult)
            V.tensor_tensor(out=M[5][:], in0=At[4][:], in1=G[5][:],
                            op=ALU.mult)
            V.tensor_tensor(out=M[6][:], in0=At[5][:], in1=G[6][:],
                            op=ALU.mult)

            # ---- B-side power ladder ----
            vmul(2, 1, 1, 0)      # w2 h0
            vmul(3, 2, 1, 0)      # w3 h0
            ssq(4, 2, 0)          # w4 h0 (Scalar)
            vmul(2, 1, 1, 1)      # w2 h1
            vmul(5, 2, 3, 0)      # w5 h0
            ssq(6, 3, 0)          # w6 h0 (Scalar)
            vmul(3, 2, 1, 1)      # w3 h1
            ssq(4, 2, 1)          # w4 h1 (Scalar)
            vmul(5, 2, 3, 1)      # w5 h1
            ssq(6, 3, 1)          # w6 h1 (Scalar)

            # ---- main accumulation: e = mask + sum_p M_p^T W_p ----
            e_ps = psE.tile([TLOC, 512], f32, name="e_ps")
            pass_order = [(1, 0), (2, 0), (1, 1), (3, 0), (2, 1), (4, 0),
                          (3, 1), (5, 0), (4, 1), (6, 0), (5, 1), (6, 1)]
            for n, (p, hc) in enumerate(pass_order):
                nc.tensor.matmul(
                    e_ps[:],
                    lhsT=M[p][:, hc * TLOC:(hc + 1) * TLOC],
                    rhs=Wt[p][:, hc * 512:(hc + 1) * 512],
                    start=(n == 0), stop=False,
                )
            nc.tensor.matmul(e_ps[:], lhsT=ones1, rhs=pm,
                             start=False, stop=True)

            # Y = enc @ WoutC^T, per s-chunk, into the freed pe_ps banks;
            # collapses the context+output GEMM into one matmul stage.
            Y_sb = bpow.tile([P, 1024], f16, name="Y_sb")
            for sc in range(4):
                y_ps = pe_ps[sc // 2][:, (sc % 2) * 256:(sc % 2 + 1) * 256]
                for kc in range(HC):
                    nc.tensor.matmul(
                        y_ps,
                        lhsT=encT[kc][:, sc * P:(sc + 1) * P],
                        rhs=wout_sb[2 + kc][:],
                        start=(kc == 0), stop=(kc == HC - 1),
                    )
                nc.vector.tensor_copy(Y_sb[:, sc * 256:(sc + 1) * 256], y_ps)

            # ---- softmax tail ----
            # exp in two 256-wide chunks; each chunk feeds 2 PE transposes,
            # a ptT copy, and 2 c^T accumulation matmuls per s-chunk.
            pt = tailp.tile([TLOC, 512], f16, name="pt")
            zacc = tailp.tile([TLOC, 1], f32, name="zacc")
            ptT_ps = psT.tile([P, 256], f16, tag="tail", name="ptT_ps")
            ptT = tailp.tile([P, 256], f16, name="ptT")
            nc.scalar.activation(pt[:], e_ps[:], AF.Exp, bias=neg4[:, 0:1],
                                 accum_out=zacc[:])
            for sb in range(4):
                nc.tensor.transpose(
                    ptT_ps[:, sb * TLOC:(sb + 1) * TLOC],
                    pt[:, sb * P:(sb + 1) * P],
                    ident[0:TLOC, 0:TLOC],
                )
            for eh in range(2):
                nc.vector.tensor_copy(ptT[:, eh * P:(eh + 1) * P],
                                      ptT_ps[:, eh * P:(eh + 1) * P])

            r_sb = tailp.tile([TLOC, 1], f32, name="r_sb")
            nc.vector.reciprocal(r_sb[:], zacc[:])

            # attn_q = q @ WoutQ^T (separate PSUM, computed off the stream
            # tail); attn_c = c^T^T @ WoutC^T; out = tanh(r*attn_c + attn_q)
            attn_q = psT.tile([TLOC, H], f32, tag="tail", name="attn_q")
            nc.tensor.matmul(attn_q[:], lhsT=qT[:, 0:TLOC],
                             rhs=wout_sb[0][:], start=True, stop=False)
            nc.tensor.matmul(attn_q[:], lhsT=qT[:, TLOC:128],
                             rhs=wout_sb[1][:], start=False, stop=True)
            attn_q_sb = tailp.tile([TLOC, H], f32, name="attn_q_sb")
            nc.vector.tensor_copy(attn_q_sb[:], attn_q[:])

            attn_c = psT.tile([TLOC, H], f32, tag="tail", name="attn_c")
            for sb in range(4):
                nc.tensor.matmul(attn_c[:],
                                 lhsT=ptT[:, sb * TLOC:(sb + 1) * TLOC],
                                 rhs=Y_sb[:, sb * 256:(sb + 1) * 256],
                                 start=(sb == 0), stop=(sb == 3))
            o_pre = tailp.tile([TLOC, H], f32, name="o_pre")
            nc.vector.scalar_tensor_tensor(
                out=o_pre[:], in0=attn_c[:], scalar=r_sb[:, 0:1],
                in1=attn_q_sb[:], op0=ALU.mult, op1=ALU.add)
            o_sb = tailp.tile([TLOC, H], f32, name="o_sb")
            for i in range(HC):
                nc.scalar.activation(o_sb[:, i * P:(i + 1) * P],
                                     o_pre[:, i * P:(i + 1) * P], AF.Tanh)
                eng = nc.sync if i == 0 else nc.scalar
                eng.dma_start(d_out[:, i * P:(i + 1) * P],
                              o_sb[:, i * P:(i + 1) * P])

    nc.compile()
    _CACHE["nc"] = nc
    return nc


def make_in_maps(query, encoder_outputs, src_lengths, Ws, Wh, v, Wout):
    h16 = np.float16
    wsT = np.asarray(Ws, h16).T
    whT = np.asarray(Wh, h16).T
    woutT = np.asarray(Wout, h16).T                  # (2H, H)
    sl = np.asarray(src_lengths)
    ident = np.eye(TLOC, dtype=h16)

    pack_a = np.zeros((NCORES, P, 1536), h16)
    pack_b = np.zeros((NCORES, P, 832), h16)
    pack_c = np.zeros((NCORES, P, 1024), h16)
    pack_d = np.zeros((NCORES, P, 1024), h16)
    pack_m = np.zeros((NCORES, 32, 512), h16)
    for c in range(NCORES):
        b, th = c // 2, c % 2
        t0 = th * TLOC
        encT = np.asarray(encoder_outputs[b], h16).T      # (H, S)
        enc = np.asarray(encoder_outputs[b], h16)         # (S, H)
        qTl = np.asarray(query[b, t0:t0 + TLOC, :], h16).T  # (H, TLOC)
        msk = (np.arange(S) < int(sl[b]))
        for kc in range(HC):
            pack_a[c, :, kc * 512:(kc + 1) * 512] = encT[kc * P:(kc + 1) * P]
            pack_a[c, :, 1024 + kc * H:1024 + (kc + 1) * H] = \
                whT[kc * P:(kc + 1) * P]
            pack_b[c, :, kc * TLOC:(kc + 1) * TLOC] = qTl[kc * P:(kc + 1) * P]
            pack_b[c, :, 128 + kc * H:128 + (kc + 1) * H] = \
                wsT[kc * P:(kc + 1) * P]
            pack_b[c, :, 640 + kc * TLOC:640 + (kc + 1) * TLOC] = \
                np.asarray(v, np.float32)[kc * P:(kc + 1) * P, None].astype(h16)
        pack_b[c, 0:TLOC, 768:832] = ident
        for sb in range(4):
            pack_c[c, :, sb * H:(sb + 1) * H] = enc[sb * P:(sb + 1) * P]
        for fc in range(4):
            pack_d[c, :, fc * H:(fc + 1) * H] = woutT[fc * P:(fc + 1) * P]
        pack_m[c, :, :] = np.where(msk, 0.0, -30.0 / 32.0)[None, :]
    return [{"pack_a": np.ascontiguousarray(pack_a[c]),
             "pack_b": np.ascontiguousarray(pack_b[c]),
             "pack_c": np.ascontiguousarray(pack_c[c]),
             "pack_d": np.ascontiguousarray(pack_d[c]),
             "pack_m": np.ascontiguousarray(pack_m[c])}
            for c in range(NCORES)]


def kernel(query, encoder_outputs, src_lengths, Ws, Wh, v, Wout):
    from concourse.bass_utils import run_bass_kernel_spmd

    nc = build_module()
    in_maps = make_in_maps(query, encoder_outputs, src_lengths, Ws, Wh, v, Wout)
    res = run_bass_kernel_spmd(nc, in_maps, core_ids=list(range(NCORES))).results
    out = np.empty((B, T, H), np.float32)
    for c in range(NCORES):
        b, th = c // 2, c % 2
        t0 = th * TLOC
        out[b, t0:t0 + TLOC, :] = res[c]["out_l"]
    return out


# revision 27
# speedup vs baseline: 1.0440x; 1.0440x over previous
"""Bahdanau additive attention on 8 Trainium2 NeuronCores.

Problem shapes (hardcoded): B=4, T=128, S=512, H=256, fp32.

Sharding: data-parallel over (batch, T-half): core c handles b = c//2,
t in [64*(c%2), 64*(c%2)+64).  Same SPMD program on every core; weights
replicated.  No collectives.

Algorithm: the additive-attention score
    e[t,s] = sum_h v[h] * tanh(pq[t,h] + pe[s,h])
is evaluated through a separable expansion instead of materializing the
(T,S,H) tensor.  With a = tanh(pq), w = tanh(pe):
    tanh(x+y) - tanh(x) ~= sum_{p=1..6} a^{p-1}(c0_p + c1_p a^2) w^p
(trimmed-minimax fit against the end-to-end output error; the tanh(x)
term is constant over s and drops under softmax shift-invariance).
Each term is a rank-1 update in (t,s) contracted over h, so e becomes
12 dense matmul passes accumulated in one PSUM bank, plus a rank-1
mask pass (single-partition ones x mask row) placed FIRST so the last
thing before exp is a data pass.  Ladder: w2=w*w, w3=w2*w, w5=w2*w3 on
Vector; w4=Sq(w2), w6=Sq(w3) on Scalar.  A-chain G_p on Vector, At/M
products on GpSimd (M1/M5/M6 on Vector to hit stream deadlines).
Inputs stream over both HWDGE DMA queues (Sync + Scalar), most-urgent
first.  Softmax tail: split exp with accum columns, PE transposes of
alpha^T, context accumulated TRANSPOSED (c^T = sum_s enc_sc^T @ ptT_sc)
so no extra c transpose is needed; out = tanh(r*attn_c + attn_q) via
one fused scalar_tensor_tensor.  fp16 throughout, fp32 PSUM.
"""

import numpy as np

B, T, S, H = 4, 128, 512, 256
TLOC = 64
NCORES = 8
P = 128
HC = H // P        # 2 h-chunks

# trimmed-minimax fit (fit_v4.py J6_123456_std): emulated p99.99 = 1.64e-2
POWERS = [1, 2, 3, 4, 5, 6]
CF = {}

_CF_KEYS = [(1, 0), (1, 2), (2, 1), (2, 3), (3, 2), (3, 4), (4, 3), (4, 5),
            (5, 4), (5, 6), (6, 5), (6, 7)]


def _load_cf():
    # coefficients produced by fit_v4.py (J6_123456_std), hardcoded
    vals = [1.024896261659279, -0.896439120239331,
            -1.2593520033723025, 0.3385575705136777,
            0.1743475243113486, -1.1750131757880733,
            2.270998990133468, 1.3712367736362034,
            4.101531713287659, -2.78572455103627,
            -8.596690258997429, 5.413507705507528]
    for (p, d), c in zip(_CF_KEYS, vals):
        CF.setdefault(p, [0.0, 0.0])
        CF[p][0 if d == p - 1 else 1] = c


_load_cf()

_CACHE = {}


def build_module():
    if "nc" in _CACHE:
        return _CACHE["nc"]

    try:
        import concourse.bass  # noqa: F401
    except ImportError:
        import sys
        sys.path.insert(0, "/opt/trn_rl_repo")

    import concourse.tile as tile
    from concourse import bacc, mybir

    f32 = mybir.dt.float32
    f16 = mybir.dt.float16
    AF = mybir.ActivationFunctionType
    ALU = mybir.AluOpType

    nc = bacc.Bacc(
        "TRN2",
        target_bir_lowering=False,
        debug=False,
        enable_asserts=False,
        num_devices=NCORES,
    )

    # packed fp16 inputs, split across the two HWDGE queues by urgency:
    #  Sync  : pk_a [encT (1024) | whT (512)]      gates pe -> ladder
    #          pk_c [ctx enc (1024)]               needed late (context)
    #  Scalar: pk_b [qT 128 | wsT 512 | v 128 | I 64]  gates pq -> A-chain
    #          pk_d [woutT (1024)]                 needed mid/late
    #  pk_m  : mask row (1, 512), rank-1 mask pass
    d_pa = nc.dram_tensor("pack_a", (P, 1536), f16, kind="ExternalInput").ap()
    d_pb = nc.dram_tensor("pack_b", (P, 832), f16, kind="ExternalInput").ap()
    d_pc = nc.dram_tensor("pack_c", (P, 1024), f16, kind="ExternalInput").ap()
    d_pd = nc.dram_tensor("pack_d", (P, 1024), f16, kind="ExternalInput").ap()
    d_pm = nc.dram_tensor("pack_m", (32, 512), f16, kind="ExternalInput").ap()
    d_out = nc.dram_tensor("out_l", (TLOC, H), f32, kind="ExternalOutput").ap()

    with tile.TileContext(nc) as tc:
        from contextlib import ExitStack

        with ExitStack() as ctx:
            consts = ctx.enter_context(tc.tile_pool(name="consts", bufs=1))
            bpow = ctx.enter_context(tc.tile_pool(name="bpow", bufs=1))
            asm = ctx.enter_context(tc.tile_pool(name="asm", bufs=1))
            tailp = ctx.enter_context(tc.tile_pool(name="tailp", bufs=1))
            psA = ctx.enter_context(tc.tile_pool(name="psA", bufs=1, space="PSUM"))
            psB = ctx.enter_context(tc.tile_pool(name="psB", bufs=1, space="PSUM"))
            psQ = ctx.enter_context(tc.tile_pool(name="psQ", bufs=1, space="PSUM"))
            psE = ctx.enter_context(tc.tile_pool(name="psE", bufs=1, space="PSUM"))
            psT = ctx.enter_context(tc.tile_pool(name="psT", bufs=3, space="PSUM"))

            pa = consts.tile([P, 1536], f16)
            nc.sync.dma_start(pa[:], d_pa[:, :])
            pb = consts.tile([P, 832], f16)
            nc.scalar.dma_start(pb[:], d_pb[:, :])
            pm_t = consts.tile([P, 512], f16)
            pm = pm_t[0:32, :]
            nc.sync.dma_start(pm, d_pm[:, :])
            pc = consts.tile([P, 1024], f16)
            nc.sync.dma_start(pc[:], d_pc[:, :])
            pd = consts.tile([P, 1024], f16)
            nc.scalar.dma_start(pd[:], d_pd[:, :])

            encT = [pa[:, 0:512], pa[:, 512:1024]]          # (h-chunk, s)
            wh_sb = [pa[:, 1024 + kc * H:1024 + (kc + 1) * H] for kc in range(HC)]
            qT = pb[:, 0:128]                               # [hc0 t | hc1 t]
            ws_sb = [pb[:, 128 + kc * H:128 + (kc + 1) * H] for kc in range(HC)]
            vbc = pb[:, 640:768]
            ident = pb[:, 768:832]                          # rows 0:64 = I64
            ctxenc = pc[:, 0:1024]                          # 4 x (128 x 256)
            wout_sb = [pd[:, fc * H:(fc + 1) * H] for fc in range(4)]

            neg4 = consts.tile([TLOC, 1], f32)
            nc.vector.memset(neg4[:], -4.0)

            ones1_t = consts.tile([P, TLOC], f16)
            ones1 = ones1_t[0:32, :]
            nc.gpsimd.memset(ones1, 1.0)

            # ---- projections (PE): pe_ps0 first (pack_a lands first),
            # then pq (pack_b), then pe_ps1 -- so tanh0 and the A-chain
            # both start as early as possible.
            pe_ps = [psA.tile([P, 512], f32, name="pe_ps0"),
                     psB.tile([P, 512], f32, name="pe_ps1")]
            pq_ps = psQ.tile([P, 128], f32, name="pq_ps")
            for kc in range(HC):
                nc.tensor.matmul(
                    pe_ps[0][:],
                    lhsT=wh_sb[kc][:, 0:P],
                    rhs=encT[kc][:],
                    start=(kc == 0), stop=(kc == HC - 1),
                )
            for oc in range(HC):
                for kc in range(HC):
                    nc.tensor.matmul(
                        pq_ps[:, oc * TLOC:(oc + 1) * TLOC],
                        lhsT=ws_sb[kc][:, oc * P:(oc + 1) * P],
                        rhs=qT[:, kc * TLOC:(kc + 1) * TLOC],
                        start=(kc == 0), stop=(kc == HC - 1),
                    )
            for kc in range(HC):
                nc.tensor.matmul(
                    pe_ps[1][:],
                    lhsT=wh_sb[kc][:, P:2 * P],
                    rhs=encT[kc][:],
                    start=(kc == 0), stop=(kc == HC - 1),
                )

            # ---- base activations ----
            alpha = asm.tile([P, 128], f16, name="alpha")
            a2 = asm.tile([P, 128], f16, name="a2")
            w1 = bpow.tile([P, 1024], f16, name="w1")
            nc.scalar.activation(w1[:, 0:512], pe_ps[0][:], AF.Tanh)
            with tc.high_priority():
                nc.scalar.activation(alpha[:], pq_ps[:], AF.Tanh)
                nc.vector.tensor_tensor(out=a2[:], in0=alpha[:], in1=alpha[:],
                                        op=ALU.mult)
            nc.scalar.activation(w1[:, 512:1024], pe_ps[1][:], AF.Tanh)

            # ---- A-side: G_p = c0 + c1 a^2 (Vector), At_k = v a^k chain
            #      (GpSimd), M_p = At_{p-1} G_p (GpSimd; 1,5,6 on Vector)
            G = {}
            M = {}
            for p in POWERS:
                G[p] = asm.tile([P, 128], f16, name=f"G{p}")
                M[p] = asm.tile([P, 128], f16, name=f"M{p}")
            At = {0: vbc}
            for k in (1, 2, 3, 4, 5):
                At[k] = asm.tile([P, 128], f16, name=f"At{k}")

            V = nc.vector
            GP = nc.gpsimd

            def mk_g(p):
                c0, c1 = CF[p]
                V.tensor_scalar(G[p][:], a2[:], float(c1), float(c0),
                                ALU.mult, ALU.add)

            with tc.high_priority():
                mk_g(1)
                V.tensor_tensor(out=M[1][:], in0=vbc[:], in1=G[1][:],
                                op=ALU.mult)
            GP.tensor_tensor(out=At[1][:], in0=vbc[:], in1=alpha[:],
                             op=ALU.mult)
            GP.tensor_tensor(out=At[2][:], in0=vbc[:], in1=a2[:], op=ALU.mult)
            Wt = {1: w1}
            for p in POWERS[1:]:
                Wt[p] = bpow.tile([P, 1024], f16, name=f"w{p}")

            def vmul(dst, sa, sb, hc):
                V.tensor_tensor(out=Wt[dst][:, hc * 512:(hc + 1) * 512],
                                in0=Wt[sa][:, hc * 512:(hc + 1) * 512],
                                in1=Wt[sb][:, hc * 512:(hc + 1) * 512],
                                op=ALU.mult)

            def ssq(dst, src, hc):
                nc.scalar.activation(Wt[dst][:, hc * 512:(hc + 1) * 512],
                                     Wt[src][:, hc * 512:(hc + 1) * 512],
                                     AF.Square)

            for p in (2, 3, 4, 5, 6):
                mk_g(p)
            GP.tensor_tensor(out=M[2][:], in0=At[1][:], in1=G[2][:],
                             op=ALU.mult)
            GP.tensor_tensor(out=At[3][:], in0=At[1][:], in1=a2[:],
                             op=ALU.mult)
            GP.tensor_tensor(out=M[3][:], in0=At[2][:], in1=G[3][:],
                             op=ALU.mult)
            GP.tensor_tensor(out=At[4][:], in0=At[2][:], in1=a2[:],
                             op=ALU.mult)
            GP.tensor_tensor(out=M[4][:], in0=At[3][:], in1=G[4][:],
                             op=ALU.mult)
            GP.tensor_tensor(out=At[5][:], in0=At[3][:], in1=a2[:],
                             op=ALU.mult)
            V.tensor_tensor(out=M[5][:], in0=At[4][:], in1=G[5][:],
                            op=ALU.mult)
            V.tensor_tensor(out=M[6][:], in0=At[5][:], in1=G[6][:],
                            op=ALU.mult)

            # ---- B-side power ladder ----
            vmul(2, 1, 1, 0)      # w2 h0
            vmul(3, 2, 1, 0)      # w3 h0
            ssq(4, 2, 0)          # w4 h0 (Scalar)
            vmul(2, 1, 1, 1)      # w2 h1
            vmul(5, 2, 3, 0)      # w5 h0
            ssq(6, 3, 0)          # w6 h0 (Scalar)
            vmul(3, 2, 1, 1)      # w3 h1
            ssq(4, 2, 1)          # w4 h1 (Scalar)
            vmul(5, 2, 3, 1)      # w5 h1
            ssq(6, 3, 1)          # w6 h1 (Scalar)

            # ---- main accumulation: e = mask + sum_p M_p^T W_p ----
            e_ps = psE.tile([TLOC, 512], f32, name="e_ps")
            pass_order = [(1, 0), (2, 0), (1, 1), (3, 0), (2, 1), (4, 0),
                          (3, 1), (5, 0), (4, 1), (6, 0), (5, 1), (6, 1)]
            for n, (p, hc) in enumerate(pass_order):
                nc.tensor.matmul(
                    e_ps[:],
                    lhsT=M[p][:, hc * TLOC:(hc + 1) * TLOC],
                    rhs=Wt[p][:, hc * 512:(hc + 1) * 512],
                    start=(n == 0), stop=False,
                )
            nc.tensor.matmul(e_ps[:], lhsT=ones1, rhs=pm,
                             start=False, stop=True)

            # Y = enc @ WoutC^T, per s-chunk, into the freed pe_ps banks;
            # collapses the context+output GEMM into one matmul stage.
            Y_sb = bpow.tile([P, 1024], f16, name="Y_sb")
            for sc in range(4):
                y_ps = pe_ps[sc // 2][:, (sc % 2) * 256:(sc % 2 + 1) * 256]
                for kc in range(HC):
                    nc.tensor.matmul(
                        y_ps,
                        lhsT=encT[kc][:, sc * P:(sc + 1) * P],
                        rhs=wout_sb[2 + kc][:],
                        start=(kc == 0), stop=(kc == HC - 1),
                    )
                nc.vector.tensor_copy(Y_sb[:, sc * 256:(sc + 1) * 256], y_ps)

            # ---- softmax tail ----
            # exp in two 256-wide chunks; each chunk feeds 2 PE transposes,
            # a ptT copy, and 2 c^T accumulation matmuls per s-chunk.
            pt = tailp.tile([TLOC, 512], f16, name="pt")
            zacc2 = tailp.tile([TLOC, 2], f32, name="zacc2")
            ptT_ps = psT.tile([P, 256], f16, tag="tail", name="ptT_ps")
            ptT = tailp.tile([P, 256], f16, name="ptT")
            for eh in range(2):
                nc.scalar.activation(pt[:, eh * 256:(eh + 1) * 256],
                                     e_ps[:, eh * 256:(eh + 1) * 256],
                                     AF.Exp, bias=neg4[:, 0:1],
                                     accum_out=zacc2[:, eh:eh + 1])
                for q2 in range(2):
                    sb = eh * 2 + q2
                    nc.tensor.transpose(
                        ptT_ps[:, sb * TLOC:(sb + 1) * TLOC],
                        pt[:, sb * P:(sb + 1) * P],
                        ident[0:TLOC, 0:TLOC],
                    )
                nc.vector.tensor_copy(ptT[:, eh * P:(eh + 1) * P],
                                      ptT_ps[:, eh * P:(eh + 1) * P])

            zacc = tailp.tile([TLOC, 1], f32, name="zacc")
            nc.vector.tensor_reduce(zacc[:], zacc2[:], mybir.AxisListType.X,
                                    ALU.add)
            r_sb = tailp.tile([TLOC, 1], f32, name="r_sb")
            nc.vector.reciprocal(r_sb[:], zacc[:])

            # attn_q = q @ WoutQ^T (separate PSUM, computed off the stream
            # tail); attn_c = c^T^T @ WoutC^T; out = tanh(r*attn_c + attn_q)
            attn_q = psT.tile([TLOC, H], f32, tag="tail", name="attn_q")
            nc.tensor.matmul(attn_q[:], lhsT=qT[:, 0:TLOC],
                             rhs=wout_sb[0][:], start=True, stop=False)
            nc.tensor.matmul(attn_q[:], lhsT=qT[:, TLOC:128],
                             rhs=wout_sb[1][:], start=False, stop=True)
            attn_q_sb = tailp.tile([TLOC, H], f32, name="attn_q_sb")
            nc.vector.tensor_copy(attn_q_sb[:], attn_q[:])

            attn_c = psT.tile([TLOC, H], f32, tag="tail", name="attn_c")
            for sb in range(4):
                nc.tensor.matmul(attn_c[:],
                                 lhsT=ptT[:, sb * TLOC:(sb + 1) * TLOC],
                                 rhs=Y_sb[:, sb * 256:(sb + 1) * 256],
                                 start=(sb == 0), stop=(sb == 3))
            o_pre = tailp.tile([TLOC, H], f32, name="o_pre")
            nc.vector.scalar_tensor_tensor(
                out=o_pre[:], in0=attn_c[:], scalar=r_sb[:, 0:1],
                in1=attn_q_sb[:], op0=ALU.mult, op1=ALU.add)
            o_sb = tailp.tile([TLOC, H], f32, name="o_sb")
            for i in range(HC):
                nc.scalar.activation(o_sb[:, i * P:(i + 1) * P],
                                     o_pre[:, i * P:(i + 1) * P], AF.Tanh)
                eng = nc.sync if i == 0 else nc.scalar
                eng.dma_start(d_out[:, i * P:(i + 1) * P],
                              o_sb[:, i * P:(i + 1) * P])

    nc.compile()
    _CACHE["nc"] = nc
    return nc


def make_in_maps(query, encoder_outputs, src_lengths, Ws, Wh, v, Wout):
    h16 = np.float16
    wsT = np.asarray(Ws, h16).T
    whT = np.asarray(Wh, h16).T
    woutT = np.asarray(Wout, h16).T                  # (2H, H)
    sl = np.asarray(src_lengths)
    ident = np.eye(TLOC, dtype=h16)

    pack_a = np.zeros((NCORES, P, 1536), h16)
    pack_b = np.zeros((NCORES, P, 832), h16)
    pack_c = np.zeros((NCORES, P, 1024), h16)
    pack_d = np.zeros((NCORES, P, 1024), h16)
    pack_m = np.zeros((NCORES, 32, 512), h16)
    for c in range(NCORES):
        b, th = c // 2, c % 2
        t0 = th * TLOC
        encT = np.asarray(encoder_outputs[b], h16).T      # (H, S)
        enc = np.asarray(encoder_outputs[b], h16)         # (S, H)
        qTl = np.asarray(query[b, t0:t0 + TLOC, :], h16).T  # (H, TLOC)
        msk = (np.arange(S) < int(sl[b]))
        for kc in range(HC):
            pack_a[c, :, kc * 512:(kc + 1) * 512] = encT[kc * P:(kc + 1) * P]
            pack_a[c, :, 1024 + kc * H:1024 + (kc + 1) * H] = \
                whT[kc * P:(kc + 1) * P]
            pack_b[c, :, kc * TLOC:(kc + 1) * TLOC] = qTl[kc * P:(kc + 1) * P]
            pack_b[c, :, 128 + kc * H:128 + (kc + 1) * H] = \
                wsT[kc * P:(kc + 1) * P]
            pack_b[c, :, 640 + kc * TLOC:640 + (kc + 1) * TLOC] = \
                np.asarray(v, np.float32)[kc * P:(kc + 1) * P, None].astype(h16)
        pack_b[c, 0:TLOC, 768:832] = ident
        for sb in range(4):
            pack_c[c, :, sb * H:(sb + 1) * H] = enc[sb * P:(sb + 1) * P]
        for fc in range(4):
            pack_d[c, :, fc * H:(fc + 1) * H] = woutT[fc * P:(fc + 1) * P]
        pack_m[c, :, :] = np.where(msk, 0.0, -30.0 / 32.0)[None, :]
    return [{"pack_a": np.ascontiguousarray(pack_a[c]),
             "pack_b": np.ascontiguousarray(pack_b[c]),
             "pack_c": np.ascontiguousarray(pack_c[c]),
             "pack_d": np.ascontiguousarray(pack_d[c]),
             "pack_m": np.ascontiguousarray(pack_m[c])}
            for c in range(NCORES)]


def kernel(query, encoder_outputs, src_lengths, Ws, Wh, v, Wout):
    from concourse.bass_utils import run_bass_kernel_spmd

    nc = build_module()
    in_maps = make_in_maps(query, encoder_outputs, src_lengths, Ws, Wh, v, Wout)
    res = run_bass_kernel_spmd(nc, in_maps, core_ids=list(range(NCORES))).results
    out = np.empty((B, T, H), np.float32)
    for c in range(NCORES):
        b, th = c // 2, c % 2
        t0 = th * TLOC
        out[b, t0:t0 + TLOC, :] = res[c]["out_l"]
    return out
